# revision 33
# baseline (speedup 1.0000x reference)
"""GATv2Conv forward on 8 Trainium2 NeuronCores (Bass/Tile).

Strategy
--------
Edges are sorted by destination node and packed into "groups" of at most
S=256 edge slots / at most 128 distinct destinations, such that every
destination's edge run lies entirely inside one group.  Groups are split
evenly across the 8 cores, so all softmax segments and output rows are
core-local: no collectives are needed.

Per core (SPMD, identical program; per-core data differs):
  - el_e = feat[src]@W_src, er_e = feat[dst]@W_dst as dense fp16 GEMMs over
    host-pre-gathered edge features (128-edge chunks, K=256 split in two).
  - score = (prelu(el+er, 0.2) * attn).sum(-1)  (ACT Prelu + mult + reduce)
  - ex = exp(score)  (batched over 8 chunks per ACT op)
  - per group: psum[rank, :] += Hx^T @ [1 | el]  where Hx[e,k] =
    (rank_e==k)*ex_e  (one-hot matmul scatter-add; ssum lands in col 0)
  - out rows = psum[:,1:257] * 1/max(ssum,tiny)  -> dense per-group rows.

Host unshards by scattering dense group rows to their global node ids
(pure index plumbing; all arithmetic happens on-device).
"""

import math
import numpy as np

import concourse.bass as bass
import concourse.mybir as mybir
import concourse.tile as tile
from concourse import bacc
from concourse.bass_utils import run_bass_kernel_spmd

F32 = mybir.dt.float32
F16 = mybir.dt.float16
BF16 = mybir.dt.bfloat16
I32 = mybir.dt.int32
AF = mybir.ActivationFunctionType
ALU = mybir.AluOpType
AX = mybir.AxisListType

N_CORES = 8
S = 256            # edge slots per group (2 chunks of 128)
CHUNK = 128
GPB = 4            # groups per batch (exp batched over 2*GPB chunks)
CPB = 2 * GPB      # chunks per batch
SC = S * GPB       # edge slots per batch
NEG_SLOPE = 0.2

LAST_RESULTS = None  # BassKernelResults of the most recent run (for test.py)
LAST_NC = None       # compiled Bacc program of the most recent run
LAST_IN_MAPS = None  # per-core input dicts of the most recent run
LAST_BUILD_ARGS = None  # (g_pc, has_bias, p1, inv_k) for rebuilds


# ----------------------------------------------------------------- host prep

def _pack_runs(counts, max_slots=S, max_nodes=128):
    """Greedily pack whole runs (same-dst edge blocks) into groups."""
    n = len(counts)
    grp = np.empty(n, np.int64)
    rank = np.empty(n, np.int64)
    g = used = nodes = 0
    for i in range(n):
        c = counts[i]
        if used + c > max_slots or nodes >= max_nodes:
            g += 1
            used = 0
            nodes = 0
        grp[i] = g
        rank[i] = nodes
        used += c
        nodes += 1
    return grp, rank, g + 1


def _prepare(feat, src, dst):
    """Sort edges by dst, pack into groups, build per-core input arrays."""
    E = dst.shape[0]
    order = np.argsort(dst, kind="stable")
    sd = dst[order].astype(np.int64)
    ss = src[order].astype(np.int64)

    uniq, counts = np.unique(sd, return_counts=True)
    # split pathological runs longer than S so packing can't fail
    need_accum = bool((counts > S).any())
    if need_accum:
        new_uniq, new_counts = [], []
        for u, c in zip(uniq, counts):
            while c > S:
                new_uniq.append(u)
                new_counts.append(S)
                c -= S
            new_uniq.append(u)
            new_counts.append(c)
        uniq = np.array(new_uniq, np.int64)
        counts = np.array(new_counts, np.int64)

    grp, rank, g_tot = _pack_runs(counts)
    n_runs = len(counts)

    starts = np.zeros(n_runs, np.int64)
    np.cumsum(counts[:-1], out=starts[1:])
    grp_first_run = np.searchsorted(grp, np.arange(g_tot))
    grp_start_edge = starts[grp_first_run]

    run_of_edge = np.repeat(np.arange(n_runs), counts)
    e_grp = grp[run_of_edge]
    e_rank = rank[run_of_edge]
    e_slot = e_grp * S + np.arange(E) - grp_start_edge[e_grp]

    g_pc = math.ceil(g_tot / N_CORES)
    g_pc = math.ceil(g_pc / GPB) * GPB          # multiple of GPB per core
    e_slots = g_pc * S
    total_slots = N_CORES * e_slots

    slot_src = np.zeros(total_slots, np.int64)
    slot_dst = np.zeros(total_slots, np.int64)
    slot_rank = np.full(total_slots, -1.0, np.float32)
    slot_src[e_slot] = ss
    slot_dst[e_slot] = sd
    slot_rank[e_slot] = e_rank

    # unshard info per run
    run_core = grp // g_pc
    run_pos = (grp % g_pc) * 128 + rank        # row in the core's dense output

    return dict(
        g_pc=g_pc, e_slots=e_slots, need_accum=need_accum,
        slot_src=slot_src, slot_dst=slot_dst, slot_rank=slot_rank,
        run_core=run_core, run_pos=run_pos, run_node=uniq,
    )


# ------------------------------------------------------------ device program

def _build_program(n_g, has_bias, p1, inv_k, repeat=1):
    """p1: number of attn>=0 dims after the host sign-sort permutation.
    inv_k: 1/K where K is the global scale folded into the W columns."""
    nc_chunks = n_g * 2
    e_slots = n_g * S
    nb = n_g // GPB
    nc = bacc.Bacc("TRN2", target_bir_lowering=False, debug=False,
                   num_devices=N_CORES)
    fsT_d = nc.dram_tensor("fsT", [2, 128, e_slots], F16, kind="ExternalInput").ap()
    fdT_d = nc.dram_tensor("fdT", [2, 128, e_slots], F16, kind="ExternalInput").ap()
    rankT_d = nc.dram_tensor("rankT", [128, nc_chunks], F32, kind="ExternalInput").ap()
    wsrc_d = nc.dram_tensor("wsrc", [2, 128, 256], F16, kind="ExternalInput").ap()
    wdst_d = nc.dram_tensor("wdst", [2, 128, 256], F16, kind="ExternalInput").ap()
    invc_d = nc.dram_tensor("invc", [1, 512], F32, kind="ExternalInput").ap()
    if has_bias:
        bsrc_d = nc.dram_tensor("bsrc", [1, 256], F16, kind="ExternalInput").ap()
        bdst_d = nc.dram_tensor("bdst", [1, 256], F16, kind="ExternalInput").ap()
    dense_d = nc.dram_tensor("dense", [n_g * 128, 256], F32,
                             kind="ExternalOutput").ap()

    with tile.TileContext(nc) as tc:
        with (
            tc.tile_pool(name="const", bufs=1) as cpool,
            tc.tile_pool(name="fsp", bufs=2) as fs_pool,
            tc.tile_pool(name="fdp", bufs=2) as fd_pool,
            tc.tile_pool(name="rkp", bufs=2) as rk_pool,
            tc.tile_pool(name="scp", bufs=2) as sc_pool,
            tc.tile_pool(name="exp", bufs=2 * GPB + 2) as ex_pool,
            tc.tile_pool(name="vp", bufs=10) as v_pool,
            tc.tile_pool(name="ep", bufs=3) as e_pool,
            tc.tile_pool(name="prp", bufs=3) as pr_pool,
            tc.tile_pool(name="hp", bufs=6) as h_pool,
            tc.tile_pool(name="smp", bufs=4) as sm_pool,
            tc.tile_pool(name="obp", bufs=3) as ob_pool,
            tc.tile_pool(name="pse", bufs=5, space="PSUM") as pse_pool,
            tc.tile_pool(name="psg", bufs=3, space="PSUM") as psg_pool,
        ):
            # ---- constants
            ws0 = cpool.tile([128, 256], F16, tag="ws0")
            ws1 = cpool.tile([128, 256], F16, tag="ws1")
            wd0 = cpool.tile([128, 256], F16, tag="wd0")
            wd1 = cpool.tile([128, 256], F16, tag="wd1")
            nc.sync.dma_start(out=ws0[:], in_=wsrc_d[0])
            nc.sync.dma_start(out=ws1[:], in_=wsrc_d[1])
            nc.sync.dma_start(out=wd0[:], in_=wdst_d[0])
            nc.sync.dma_start(out=wd1[:], in_=wdst_d[1])
            invc_b = cpool.tile([128, 512], F32, tag="invcb")
            nc.gpsimd.dma_start(out=invc_b[:], in_=invc_d[:].to_broadcast((128, 512)))
            iota_i = cpool.tile([128, 128], I32, tag="iotai")
            nc.gpsimd.iota(iota_i[:], [[1, 128]], channel_multiplier=0)
            iota_f = cpool.tile([128, 128], F32, tag="iotaf")
            nc.vector.tensor_copy(iota_f[:], iota_i[:])
            if has_bias:
                ones1 = cpool.tile([1, 128], F16, tag="ones1")
                nc.gpsimd.memset(ones1[:], 1.0)
                bs_sb = cpool.tile([1, 256], F16, tag="bs")
                bd_sb = cpool.tile([1, 256], F16, tag="bd")
                nc.sync.dma_start(out=bs_sb[:], in_=bsrc_d[:])
                nc.sync.dma_start(out=bd_sb[:], in_=bdst_d[:])

            import contextlib
            _rep = contextlib.ExitStack()
            if repeat > 1:
                _rep.enter_context(tc.For_i(0, repeat, 1))
            for b in range(nb):
                fs = fs_pool.tile([128, 2, SC], F16, tag="fs")
                fd = fd_pool.tile([128, 2, SC], F16, tag="fd")
                nc.sync.dma_start(
                    out=fs[:], in_=fsT_d[:, :, b * SC:(b + 1) * SC]
                    .rearrange("k p e -> p k e"))
                nc.sync.dma_start(
                    out=fd[:], in_=fdT_d[:, :, b * SC:(b + 1) * SC]
                    .rearrange("k p e -> p k e"))
                fs0, fs1 = fs[:, 0], fs[:, 1]
                fd0, fd1 = fd[:, 0], fd[:, 1]
                rk = rk_pool.tile([128, CPB], F32, tag="rk")
                nc.sync.dma_start(out=rk[:], in_=rankT_d[:, b * CPB:(b + 1) * CPB])
                sc_col = sc_pool.tile([128, CPB], F32, tag="sc")
                v2s = []
                for mac in range(GPB):     # one macro = one group = 2 chunks
                    pe2 = pse_pool.tile([128, 512], F32, tag="pe2")  # one bank
                    v2 = v_pool.tile([128, 514], BF16, tag="v2")
                    v2r = v2[:].rearrange("p (g c) -> p g c", c=257)
                    nc.gpsimd.memset(v2r[:, :, 0:1], 1.0)
                    last_er = None
                    for m in range(2):
                        # chunk m uses cols [m*256, m*256+256) of the bank.
                        # Its el MMs + el evacuation + er MMs all complete
                        # before chunk m+1's start=True clears the bank's
                        # has_written bits (values are untouched by that).
                        j = mac * 2 + m
                        s0, s1 = j * CHUNK, (j + 1) * CHUNK
                        o = pe2[:, m * 256:(m + 1) * 256]
                        mm0 = nc.tensor.matmul(out=o, lhsT=fs0[:, s0:s1],
                                               rhs=ws0[:], start=True, stop=False)
                        if last_er is not None:
                            # start=True clears has_written for the whole bank:
                            # must not be reordered before chunk m-1's er MMs.
                            tile.add_dep_helper(
                                mm0.ins, last_er.ins, sync=False,
                                reason="bank has_written ordering")
                        nc.tensor.matmul(out=o, lhsT=fs1[:, s0:s1], rhs=ws1[:],
                                         start=False, stop=False)
                        if has_bias:
                            nc.tensor.matmul(out=o, lhsT=ones1[:], rhs=bs_sb[:],
                                             start=False, stop=False)
                        # el = s~ * (1/c)  (recovers the un-folded projection)
                        nc.vector.tensor_tensor(
                            out=v2r[:, m, 1:257], in0=o,
                            in1=invc_b[:, 0:256], op=ALU.mult)
                        nc.tensor.matmul(out=o, lhsT=fd0[:, s0:s1], rhs=wd0[:],
                                         start=False, stop=False)
                        last_er = nc.tensor.matmul(out=o, lhsT=fd1[:, s0:s1],
                                                   rhs=wd1[:], start=False,
                                                   stop=not has_bias)
                        if has_bias:
                            last_er = nc.tensor.matmul(
                                out=o, lhsT=ones1[:], rhs=bd_sb[:],
                                start=False, stop=True)
                    e2 = e_pool.tile([128, 512], BF16, tag="e2")
                    e2r = e2[:].rearrange("p (g c) -> p g c", c=256)
                    nc.scalar.activation(e2[:], pe2[:], AF.Prelu,
                                         alpha=NEG_SLOPE)
                    # score = sum(e~ over attn>=0 dims) - sum(e~ over attn<0)
                    sc_sl = sc_col[:, 2 * mac:2 * mac + 2]
                    if p1 > 0:
                        nc.vector.tensor_reduce(out=sc_sl, in_=e2r[:, :, 0:p1],
                                                axis=AX.X, op=ALU.add)
                    if p1 < 256:
                        rn = pr_pool.tile([128, 2], F32, tag="rn")
                        nc.vector.tensor_reduce(out=rn[:], in_=e2r[:, :, p1:256],
                                                axis=AX.X, op=ALU.add)
                        if p1 > 0:
                            nc.gpsimd.tensor_tensor(out=sc_sl, in0=sc_sl,
                                                    in1=rn[:], op=ALU.subtract)
                        else:
                            nc.gpsimd.tensor_scalar(
                                out=sc_sl, in0=rn[:], scalar1=-1.0, scalar2=None,
                                op0=ALU.mult)
                    exv = ex_pool.tile([128, 2], F32, tag="ex")
                    nc.scalar.activation(exv[:], sc_sl, AF.Exp, scale=inv_k)
                    v2s.append((v2, exv))
                ob4 = ob_pool.tile([128, GPB * 256], F32, tag="ob4")
                for gl in range(GPB):
                    v2, exv = v2s[gl]
                    pg = psg_pool.tile([128, 257], F32, tag="pg")
                    for m in range(2):
                        j = gl * 2 + m
                        hx = h_pool.tile([128, 128], BF16, tag="hx")
                        nc.gpsimd.tensor_scalar(
                            out=hx[:], in0=iota_f[:], scalar1=rk[:, j:j + 1],
                            scalar2=exv[:, m:m + 1],
                            op0=ALU.is_equal, op1=ALU.mult)
                        nc.tensor.matmul(out=pg[:], lhsT=hx[:],
                                         rhs=v2[:, m * 257:(m + 1) * 257],
                                         start=(m == 0), stop=(m == 1))
                    ssum = sm_pool.tile([128, 1], F32, tag="ssum")
                    nc.vector.tensor_scalar_max(out=ssum[:], in0=pg[:, 0:1],
                                                scalar1=1e-30)
                    rcp = sm_pool.tile([128, 1], F32, tag="rcp")
                    nc.vector.reciprocal_approx_fast(rcp[:], ssum[:])
                    nc.scalar.mul(ob4[:, gl * 256:(gl + 1) * 256],
                                  pg[:, 1:257], rcp[:, 0:1])
                g0 = b * GPB
                nc.sync.dma_start(
                    out=dense_d[g0 * 128:(g0 + GPB) * 128, :]
                    .rearrange("(g r) c -> r g c", r=128),
                    in_=ob4[:].rearrange("p (g c) -> p g c", c=256))
            _rep.close()
    nc.compile()
    return nc


# ------------------------------------------------------------------- kernel

def kernel(feat, W_src, b_src, W_dst, b_dst, attn, src, dst, _trace=False):
    global LAST_RESULTS, LAST_NC, LAST_IN_MAPS
    feat = np.asarray(feat, np.float32)
    n_nodes, d_in = feat.shape
    d_out = W_src.shape[1]
    assert d_in == 256 and d_out == 256, "kernel is specialized to D=256"

    p = _prepare(feat, np.asarray(src), np.asarray(dst))
    g_pc, e_slots = p["g_pc"], p["e_slots"]

    has_bias = bool(np.any(b_src) or np.any(b_dst))

    # Fold |attn| (sign-sorted col-permutation, global scale K) into W columns:
    #   psum = fs@W~s + fd@W~d = c_d * (el+er)_d, prelu -> e~ = c_d*lrelu(...)
    #   score*K = sum(e~[P]) - sum(e~[N]);  el = psum_el * (1/c)
    attn_f = np.asarray(attn, np.float32).reshape(256)
    perm = np.argsort(attn_f < 0, kind="stable")
    p1 = int((attn_f >= 0).sum())
    inv_perm = np.argsort(perm)
    a_perm = attn_f[perm]
    K = float(np.clip(0.02 / max(np.abs(attn_f).min(), 1e-7), 1.0, 1e5))
    c = K * np.maximum(np.abs(a_perm), 1e-7)
    global LAST_BUILD_ARGS
    LAST_BUILD_ARGS = (g_pc, has_bias, p1, 1.0 / K)
    nc = _build_program(g_pc, has_bias, p1, 1.0 / K)

    feat16 = feat.astype(np.float16)
    wsrc_f = np.asarray(W_src, np.float32)[:, perm] * c[None, :]
    wdst_f = np.asarray(W_dst, np.float32)[:, perm] * c[None, :]
    wsrc16 = np.ascontiguousarray(wsrc_f.astype(np.float16).reshape(2, 128, 256))
    wdst16 = np.ascontiguousarray(wdst_f.astype(np.float16).reshape(2, 128, 256))
    invc_in = np.ascontiguousarray(np.tile(1.0 / c, 2).reshape(1, 512)
                                   .astype(np.float32))

    in_maps = []
    for ci in range(N_CORES):
        sl = slice(ci * e_slots, (ci + 1) * e_slots)
        fs = feat16[p["slot_src"][sl]]          # [e_slots, 256] f16
        fd = feat16[p["slot_dst"][sl]]
        fsT = np.ascontiguousarray(fs.T).reshape(2, 128, e_slots)
        fdT = np.ascontiguousarray(fd.T).reshape(2, 128, e_slots)
        rankT = np.ascontiguousarray(
            p["slot_rank"][sl].reshape(g_pc * 2, 128).T)
        m = {"fsT": fsT, "fdT": fdT, "rankT": rankT,
             "wsrc": wsrc16, "wdst": wdst16, "invc": invc_in}
        if has_bias:
            m["bsrc"] = np.ascontiguousarray(
                (np.asarray(b_src, np.float32)[perm] * c)
                .astype(np.float16).reshape(1, 256))
            m["bdst"] = np.ascontiguousarray(
                (np.asarray(b_dst, np.float32)[perm] * c)
                .astype(np.float16).reshape(1, 256))
        in_maps.append(m)

    res = run_bass_kernel_spmd(nc, in_maps, core_ids=list(range(N_CORES)),
                               trace=_trace)
    LAST_RESULTS, LAST_NC, LAST_IN_MAPS = res, nc, in_maps

    out = np.zeros((n_nodes, 256), np.float32)
    run_core, run_pos, run_node = p["run_core"], p["run_pos"], p["run_node"]
    for ci in range(N_CORES):
        dense = res.results[ci]["dense"]
        mask = run_core == ci
        if not mask.any():
            continue
        rows = dense[run_pos[mask]][:, inv_perm]   # undo the attn column sort
        if p["need_accum"]:
            np.add.at(out, run_node[mask], rows)
        else:
            out[run_node[mask]] = rows
    return out


# revision 35
# speedup vs baseline: 2.4710x; 2.4710x over previous
"""GATv2Conv forward on 8 Trainium2 NeuronCores (Bass/Tile).

Strategy
--------
Edges are sorted by destination node and packed into "groups" of at most
S=256 edge slots / at most 128 distinct destinations, such that every
destination's edge run lies entirely inside one group.  Groups are split
evenly across the 8 cores, so all softmax segments and output rows are
core-local: no collectives are needed.

Per core (SPMD, identical program; per-core data differs):
  - el_e = feat[src]@W_src, er_e = feat[dst]@W_dst as dense fp16 GEMMs over
    host-pre-gathered edge features (128-edge chunks, K=256 split in two).
  - score = (prelu(el+er, 0.2) * attn).sum(-1)  (ACT Prelu + mult + reduce)
  - ex = exp(score)  (batched over 8 chunks per ACT op)
  - per group: psum[rank, :] += Hx^T @ [1 | el]  where Hx[e,k] =
    (rank_e==k)*ex_e  (one-hot matmul scatter-add; ssum lands in col 0)
  - out rows = psum[:,1:257] * 1/max(ssum,tiny)  -> dense per-group rows.

Host unshards by scattering dense group rows to their global node ids
(pure index plumbing; all arithmetic happens on-device).
"""

import math
import numpy as np

import concourse.bass as bass
import concourse.mybir as mybir
import concourse.tile as tile
from concourse import bacc
from concourse.bass_utils import run_bass_kernel_spmd

F32 = mybir.dt.float32
F16 = mybir.dt.float16
BF16 = mybir.dt.bfloat16
I32 = mybir.dt.int32
AF = mybir.ActivationFunctionType
ALU = mybir.AluOpType
AX = mybir.AxisListType

N_CORES = 8
S = 256            # edge slots per group (2 chunks of 128)
CHUNK = 128
GPB = 4            # groups per batch (exp batched over 2*GPB chunks)
CPB = 2 * GPB      # chunks per batch
SC = S * GPB       # edge slots per batch
NEG_SLOPE = 0.2

LAST_RESULTS = None  # BassKernelResults of the most recent run (for test.py)
LAST_NC = None       # compiled Bacc program of the most recent run
LAST_IN_MAPS = None  # per-core input dicts of the most recent run
LAST_BUILD_ARGS = None  # (g_pc, has_bias, p1, inv_k) for rebuilds


# ----------------------------------------------------------------- host prep

def _pack_runs(counts, max_slots=S, max_nodes=128):
    """Greedily pack whole runs (same-dst edge blocks) into groups."""
    n = len(counts)
    grp = np.empty(n, np.int64)
    rank = np.empty(n, np.int64)
    g = used = nodes = 0
    for i in range(n):
        c = counts[i]
        if used + c > max_slots or nodes >= max_nodes:
            g += 1
            used = 0
            nodes = 0
        grp[i] = g
        rank[i] = nodes
        used += c
        nodes += 1
    return grp, rank, g + 1


def _prepare(feat, src, dst):
    """Sort edges by dst, pack into groups, build per-core input arrays."""
    E = dst.shape[0]
    order = np.argsort(dst, kind="stable")
    sd = dst[order].astype(np.int64)
    ss = src[order].astype(np.int64)

    uniq, counts = np.unique(sd, return_counts=True)
    # split pathological runs longer than S so packing can't fail
    need_accum = bool((counts > S).any())
    if need_accum:
        new_uniq, new_counts = [], []
        for u, c in zip(uniq, counts):
            while c > S:
                new_uniq.append(u)
                new_counts.append(S)
                c -= S
            new_uniq.append(u)
            new_counts.append(c)
        uniq = np.array(new_uniq, np.int64)
        counts = np.array(new_counts, np.int64)

    grp, rank, g_tot = _pack_runs(counts)
    n_runs = len(counts)

    starts = np.zeros(n_runs, np.int64)
    np.cumsum(counts[:-1], out=starts[1:])
    grp_first_run = np.searchsorted(grp, np.arange(g_tot))
    grp_start_edge = starts[grp_first_run]

    run_of_edge = np.repeat(np.arange(n_runs), counts)
    e_grp = grp[run_of_edge]
    e_rank = rank[run_of_edge]
    e_slot = e_grp * S + np.arange(E) - grp_start_edge[e_grp]

    g_pc = math.ceil(g_tot / N_CORES)
    g_pc = math.ceil(g_pc / GPB) * GPB          # multiple of GPB per core
    e_slots = g_pc * S
    total_slots = N_CORES * e_slots

    slot_src = np.zeros(total_slots, np.int64)
    slot_dst = np.zeros(total_slots, np.int64)
    slot_rank = np.full(total_slots, -1.0, np.float32)
    slot_src[e_slot] = ss
    slot_dst[e_slot] = sd
    slot_rank[e_slot] = e_rank

    # unshard info per run
    run_core = grp // g_pc
    run_pos = (grp % g_pc) * 128 + rank        # row in the core's dense output

    return dict(
        g_pc=g_pc, e_slots=e_slots, need_accum=need_accum,
        slot_src=slot_src, slot_dst=slot_dst, slot_rank=slot_rank,
        run_core=run_core, run_pos=run_pos, run_node=uniq,
    )


# ------------------------------------------------------------ device program

def _build_program(n_g, has_bias, p1, inv_k, repeat=1):
    """p1: number of attn>=0 dims after the host sign-sort permutation.
    inv_k: 1/K where K is the global scale folded into the W columns."""
    nc_chunks = n_g * 2
    e_slots = n_g * S
    nb = n_g // GPB
    nc = bacc.Bacc("TRN2", target_bir_lowering=False, debug=False,
                   num_devices=N_CORES)
    fsT_d = nc.dram_tensor("fsT", [2, 128, e_slots], F16, kind="ExternalInput").ap()
    fdT_d = nc.dram_tensor("fdT", [2, 128, e_slots], F16, kind="ExternalInput").ap()
    rankT_d = nc.dram_tensor("rankT", [128, nc_chunks], F32, kind="ExternalInput").ap()
    wsrc_d = nc.dram_tensor("wsrc", [2, 128, 256], F16, kind="ExternalInput").ap()
    wdst_d = nc.dram_tensor("wdst", [2, 128, 256], F16, kind="ExternalInput").ap()
    invc_d = nc.dram_tensor("invc", [1, 512], F32, kind="ExternalInput").ap()
    if has_bias:
        bsrc_d = nc.dram_tensor("bsrc", [1, 256], F16, kind="ExternalInput").ap()
        bdst_d = nc.dram_tensor("bdst", [1, 256], F16, kind="ExternalInput").ap()
    dense_d = nc.dram_tensor("dense", [n_g * 128, 256], F32,
                             kind="ExternalOutput").ap()

    with tile.TileContext(nc) as tc:
        with (
            tc.tile_pool(name="const", bufs=1) as cpool,
            tc.tile_pool(name="fsp", bufs=2) as fs_pool,
            tc.tile_pool(name="fdp", bufs=2) as fd_pool,
            tc.tile_pool(name="rkp", bufs=2) as rk_pool,
            tc.tile_pool(name="scp", bufs=2) as sc_pool,
            tc.tile_pool(name="exp", bufs=2 * GPB + 2) as ex_pool,
            tc.tile_pool(name="vp", bufs=10) as v_pool,
            tc.tile_pool(name="ep", bufs=3) as e_pool,
            tc.tile_pool(name="prp", bufs=3) as pr_pool,
            tc.tile_pool(name="hp", bufs=6) as h_pool,
            tc.tile_pool(name="smp", bufs=4) as sm_pool,
            tc.tile_pool(name="obp", bufs=3) as ob_pool,
            tc.tile_pool(name="pse", bufs=5, space="PSUM") as pse_pool,
            tc.tile_pool(name="psg", bufs=3, space="PSUM") as psg_pool,
        ):
            # ---- constants
            ws0 = cpool.tile([128, 256], F16, tag="ws0")
            ws1 = cpool.tile([128, 256], F16, tag="ws1")
            wd0 = cpool.tile([128, 256], F16, tag="wd0")
            wd1 = cpool.tile([128, 256], F16, tag="wd1")
            nc.sync.dma_start(out=ws0[:], in_=wsrc_d[0])
            nc.sync.dma_start(out=ws1[:], in_=wsrc_d[1])
            nc.sync.dma_start(out=wd0[:], in_=wdst_d[0])
            nc.sync.dma_start(out=wd1[:], in_=wdst_d[1])
            invc_b = cpool.tile([128, 512], F32, tag="invcb")
            nc.gpsimd.dma_start(out=invc_b[:], in_=invc_d[:].to_broadcast((128, 512)))
            iota_i = cpool.tile([128, 128], I32, tag="iotai")
            nc.gpsimd.iota(iota_i[:], [[1, 128]], channel_multiplier=0)
            iota_f = cpool.tile([128, 128], F32, tag="iotaf")
            nc.vector.tensor_copy(iota_f[:], iota_i[:])
            if has_bias:
                ones1 = cpool.tile([1, 128], F16, tag="ones1")
                nc.gpsimd.memset(ones1[:], 1.0)
                bs_sb = cpool.tile([1, 256], F16, tag="bs")
                bd_sb = cpool.tile([1, 256], F16, tag="bd")
                nc.sync.dma_start(out=bs_sb[:], in_=bsrc_d[:])
                nc.sync.dma_start(out=bd_sb[:], in_=bdst_d[:])

            import contextlib
            _rep = contextlib.ExitStack()
            if repeat > 1:
                _rep.enter_context(tc.For_i(0, repeat, 1))
            for b in range(nb):
                fs = fs_pool.tile([128, 2, SC], F16, tag="fs")
                fd = fd_pool.tile([128, 2, SC], F16, tag="fd")
                nc.sync.dma_start(
                    out=fs[:], in_=fsT_d[:, :, b * SC:(b + 1) * SC]
                    .rearrange("k p e -> p k e"))
                nc.sync.dma_start(
                    out=fd[:], in_=fdT_d[:, :, b * SC:(b + 1) * SC]
                    .rearrange("k p e -> p k e"))
                fs0, fs1 = fs[:, 0], fs[:, 1]
                fd0, fd1 = fd[:, 0], fd[:, 1]
                rk = rk_pool.tile([128, CPB], F32, tag="rk")
                nc.sync.dma_start(out=rk[:], in_=rankT_d[:, b * CPB:(b + 1) * CPB])
                sc_col = sc_pool.tile([128, CPB], F32, tag="sc")
                v2s = []
                for mac in range(GPB):     # one macro = one group = 2 chunks
                    pe2 = pse_pool.tile([128, 512], F32, tag="pe2")  # one bank
                    v2 = v_pool.tile([128, 514], BF16, tag="v2")
                    v2r = v2[:].rearrange("p (g c) -> p g c", c=257)
                    nc.vector.memset(v2r[:, :, 0:1], 1.0)
                    last_er = None
                    for m in range(2):
                        # chunk m uses cols [m*256, m*256+256) of the bank.
                        # Its el MMs + el evacuation + er MMs all complete
                        # before chunk m+1's start=True clears the bank's
                        # has_written bits (values are untouched by that).
                        j = mac * 2 + m
                        s0, s1 = j * CHUNK, (j + 1) * CHUNK
                        o = pe2[:, m * 256:(m + 1) * 256]
                        mm0 = nc.tensor.matmul(out=o, lhsT=fs0[:, s0:s1],
                                               rhs=ws0[:], start=True, stop=False)
                        if last_er is not None:
                            # start=True clears has_written for the whole bank:
                            # must not be reordered before chunk m-1's er MMs.
                            tile.add_dep_helper(
                                mm0.ins, last_er.ins, sync=False,
                                reason="bank has_written ordering")
                        nc.tensor.matmul(out=o, lhsT=fs1[:, s0:s1], rhs=ws1[:],
                                         start=False, stop=False)
                        if has_bias:
                            nc.tensor.matmul(out=o, lhsT=ones1[:], rhs=bs_sb[:],
                                             start=False, stop=False)
                        # el = s~ * (1/c)  (recovers the un-folded projection)
                        nc.vector.tensor_tensor(
                            out=v2r[:, m, 1:257], in0=o,
                            in1=invc_b[:, 0:256], op=ALU.mult)
                        nc.tensor.matmul(out=o, lhsT=fd0[:, s0:s1], rhs=wd0[:],
                                         start=False, stop=False)
                        last_er = nc.tensor.matmul(out=o, lhsT=fd1[:, s0:s1],
                                                   rhs=wd1[:], start=False,
                                                   stop=not has_bias)
                        if has_bias:
                            last_er = nc.tensor.matmul(
                                out=o, lhsT=ones1[:], rhs=bd_sb[:],
                                start=False, stop=True)
                    e2 = e_pool.tile([128, 512], BF16, tag="e2")
                    e2r = e2[:].rearrange("p (g c) -> p g c", c=256)
                    nc.scalar.activation(e2[:], pe2[:], AF.Prelu,
                                         alpha=NEG_SLOPE)
                    # score = sum(e~ over attn>=0 dims) - sum(e~ over attn<0)
                    sc_sl = sc_col[:, 2 * mac:2 * mac + 2]
                    if p1 > 0:
                        nc.vector.tensor_reduce(out=sc_sl, in_=e2r[:, :, 0:p1],
                                                axis=AX.X, op=ALU.add)
                    if p1 < 256:
                        rn = pr_pool.tile([128, 2], F32, tag="rn")
                        nc.vector.tensor_reduce(out=rn[:], in_=e2r[:, :, p1:256],
                                                axis=AX.X, op=ALU.add)
                        if p1 > 0:
                            nc.vector.tensor_tensor(out=sc_sl, in0=sc_sl,
                                                    in1=rn[:], op=ALU.subtract)
                        else:
                            nc.vector.tensor_scalar(
                                out=sc_sl, in0=rn[:], scalar1=-1.0, scalar2=None,
                                op0=ALU.mult)
                    exv = ex_pool.tile([128, 2], F32, tag="ex")
                    nc.scalar.activation(exv[:], sc_sl, AF.Exp, scale=inv_k)
                    v2s.append((v2, exv))
                ob4 = ob_pool.tile([128, GPB * 256], F32, tag="ob4")
                for gl in range(GPB):
                    v2, exv = v2s[gl]
                    pg = psg_pool.tile([128, 257], F32, tag="pg")
                    for m in range(2):
                        j = gl * 2 + m
                        hx = h_pool.tile([128, 128], BF16, tag="hx")
                        nc.vector.tensor_scalar(
                            out=hx[:], in0=iota_f[:], scalar1=rk[:, j:j + 1],
                            scalar2=exv[:, m:m + 1],
                            op0=ALU.is_equal, op1=ALU.mult)
                        nc.tensor.matmul(out=pg[:], lhsT=hx[:],
                                         rhs=v2[:, m * 257:(m + 1) * 257],
                                         start=(m == 0), stop=(m == 1))
                    ssum = sm_pool.tile([128, 1], F32, tag="ssum")
                    nc.vector.tensor_scalar_max(out=ssum[:], in0=pg[:, 0:1],
                                                scalar1=1e-30)
                    rcp = sm_pool.tile([128, 1], F32, tag="rcp")
                    nc.vector.reciprocal_approx_fast(rcp[:], ssum[:])
                    nc.scalar.mul(ob4[:, gl * 256:(gl + 1) * 256],
                                  pg[:, 1:257], rcp[:, 0:1])
                g0 = b * GPB
                nc.sync.dma_start(
                    out=dense_d[g0 * 128:(g0 + GPB) * 128, :]
                    .rearrange("(g r) c -> r g c", r=128),
                    in_=ob4[:].rearrange("p (g c) -> p g c", c=256))
            _rep.close()
    nc.compile()
    return nc


# ------------------------------------------------------------------- kernel

def kernel(feat, W_src, b_src, W_dst, b_dst, attn, src, dst, _trace=False):
    global LAST_RESULTS, LAST_NC, LAST_IN_MAPS
    feat = np.asarray(feat, np.float32)
    n_nodes, d_in = feat.shape
    d_out = W_src.shape[1]
    assert d_in == 256 and d_out == 256, "kernel is specialized to D=256"

    p = _prepare(feat, np.asarray(src), np.asarray(dst))
    g_pc, e_slots = p["g_pc"], p["e_slots"]

    has_bias = bool(np.any(b_src) or np.any(b_dst))

    # Fold |attn| (sign-sorted col-permutation, global scale K) into W columns:
    #   psum = fs@W~s + fd@W~d = c_d * (el+er)_d, prelu -> e~ = c_d*lrelu(...)
    #   score*K = sum(e~[P]) - sum(e~[N]);  el = psum_el * (1/c)
    attn_f = np.asarray(attn, np.float32).reshape(256)
    perm = np.argsort(attn_f < 0, kind="stable")
    p1 = int((attn_f >= 0).sum())
    inv_perm = np.argsort(perm)
    a_perm = attn_f[perm]
    K = float(np.clip(0.02 / max(np.abs(attn_f).min(), 1e-7), 1.0, 1e5))
    c = K * np.maximum(np.abs(a_perm), 1e-7)
    global LAST_BUILD_ARGS
    LAST_BUILD_ARGS = (g_pc, has_bias, p1, 1.0 / K)
    nc = _build_program(g_pc, has_bias, p1, 1.0 / K)

    feat16 = feat.astype(np.float16)
    wsrc_f = np.asarray(W_src, np.float32)[:, perm] * c[None, :]
    wdst_f = np.asarray(W_dst, np.float32)[:, perm] * c[None, :]
    wsrc16 = np.ascontiguousarray(wsrc_f.astype(np.float16).reshape(2, 128, 256))
    wdst16 = np.ascontiguousarray(wdst_f.astype(np.float16).reshape(2, 128, 256))
    invc_in = np.ascontiguousarray(np.tile(1.0 / c, 2).reshape(1, 512)
                                   .astype(np.float32))

    in_maps = []
    for ci in range(N_CORES):
        sl = slice(ci * e_slots, (ci + 1) * e_slots)
        fs = feat16[p["slot_src"][sl]]          # [e_slots, 256] f16
        fd = feat16[p["slot_dst"][sl]]
        fsT = np.ascontiguousarray(fs.T).reshape(2, 128, e_slots)
        fdT = np.ascontiguousarray(fd.T).reshape(2, 128, e_slots)
        rankT = np.ascontiguousarray(
            p["slot_rank"][sl].reshape(g_pc * 2, 128).T)
        m = {"fsT": fsT, "fdT": fdT, "rankT": rankT,
             "wsrc": wsrc16, "wdst": wdst16, "invc": invc_in}
        if has_bias:
            m["bsrc"] = np.ascontiguousarray(
                (np.asarray(b_src, np.float32)[perm] * c)
                .astype(np.float16).reshape(1, 256))
            m["bdst"] = np.ascontiguousarray(
                (np.asarray(b_dst, np.float32)[perm] * c)
                .astype(np.float16).reshape(1, 256))
        in_maps.append(m)

    res = run_bass_kernel_spmd(nc, in_maps, core_ids=list(range(N_CORES)),
                               trace=_trace)
    LAST_RESULTS, LAST_NC, LAST_IN_MAPS = res, nc, in_maps

    out = np.zeros((n_nodes, 256), np.float32)
    run_core, run_pos, run_node = p["run_core"], p["run_pos"], p["run_node"]
    for ci in range(N_CORES):
        dense = res.results[ci]["dense"]
        mask = run_core == ci
        if not mask.any():
            continue
        rows = dense[run_pos[mask]][:, inv_perm]   # undo the attn column sort
        if p["need_accum"]:
            np.add.at(out, run_node[mask], rows)
        else:
            out[run_node[mask]] = rows
    return out
